# revision 19
# baseline (speedup 1.0000x reference)
"""MultiHeadCrossAttention on 8 TRN2 NeuronCores.

Sharding: tensor-parallel over heads (16 heads -> 2 per core).
All matmul inputs are fp16 (host-converted), halving HBM traffic; PSUM
accumulation stays fp32. Bias algebra: bk vanishes under softmax (it only
adds a per-query constant), bv folds into the output bias on host
(softmax rows sum to 1), so only bq is applied on device.
Per core:
  Q.T = (Wq.T slice).T @ x1.T + bq     [128, 4096]  (features x tokens)
  K.T from x2.T                        [128, 8192]
  V    projected directly in [kv, d] natural layout (lhsT = x2.T chunk),
       stored per 128-kv chunk as [kv, 2*(64+1)] with a ones column that
       yields the softmax denominator for free.
  scores phase (b,g,h,kc): S.T = K @ Q.T in PSUM; P.T = exp(S.T/8) fp16,
       buffered in SBUF for the whole (b,g) block.
  consumer phase: O[q, d|den] = sum_kv P.T-chunk.T @ V-chunk, one
       CONTIGUOUS accumulation group per PSUM bank (matmul start= clears
       the whole bank's has_written bits, so groups must not interleave
       within a bank); normalize A = O[:, :64]*recip(O[:, 64]);
       PE-transpose -> A.T ; Y.T partial = (Wo.T row-slice).T @ A.T.
Macro-pipeline: phase (b,g)'s consumers are emitted interleaved into
phase (b,g)+1's scores/exp stream so the Act engine (exp) and PE stay
concurrently busy; projections of batch b+1 are chopped into quanta and
interleaved the same way. Host: pre-tiles/converts inputs, sums the 8
fp16 partials in fp32, adds bo_eff = bo + Wo @ bv, transposes back.
"""
import numpy as np
from contextlib import ExitStack

import concourse.bass as bass
import concourse.mybir as mybir
import concourse.tile as tile
from concourse import bacc
from concourse.bass_utils import run_bass_kernel_spmd

N_CORES = 8
B, SQ, SKV, E, DH = 4, 1024, 2048, 1024, 64
Q_ROWS = B * SQ      # 4096
KV_ROWS = B * SKV    # 8192
EC = E // 128        # 8 contraction chunks
QC = Q_ROWS // 512   # 8 q column chunks
KVC_B = SKV // 128   # 16 kv chunks per batch
F16 = mybir.dt.float16
F32 = mybir.dt.float32
Exp = mybir.ActivationFunctionType.Exp

_CACHE = {}


def _build(phases=("proj", "attn", "oproj"), n_reps=1):
    nc = bacc.Bacc("TRN2", target_bir_lowering=False, debug=False,
                   num_devices=N_CORES)
    x1t = nc.dram_tensor("x1t", [QC, 128, EC, 512], F16,
                         kind="ExternalInput").ap()
    x2t = nc.dram_tensor("x2t", [KV_ROWS // 512, 128, EC, 512], F16,
                         kind="ExternalInput").ap()
    wqt = nc.dram_tensor("wqt", [128, EC, 128], F16, kind="ExternalInput").ap()
    wkt = nc.dram_tensor("wkt", [128, EC, 128], F16, kind="ExternalInput").ap()
    wvt = nc.dram_tensor("wvt", [128, EC, 128], F16, kind="ExternalInput").ap()
    wot = nc.dram_tensor("wot", [128, E], F16, kind="ExternalInput").ap()
    bqv = nc.dram_tensor("bq", [128, 1], F32, kind="ExternalInput").ap()
    idv = nc.dram_tensor("ident", [128, 128], F16, kind="ExternalInput").ap()
    yt = nc.dram_tensor("yt", [E, Q_ROWS], F16, kind="ExternalOutput").ap()
    yt_r = yt.rearrange("(oc p) q -> p oc q", p=128)

    do_proj = "proj" in phases
    do_attn = "attn" in phases and do_proj
    do_oproj = "oproj" in phases and do_attn

    with tile.TileContext(nc) as tc, ExitStack() as ctx:
        const = ctx.enter_context(tc.tile_pool(name="const", bufs=1))
        persist = ctx.enter_context(tc.tile_pool(name="persist", bufs=1))
        xload = ctx.enter_context(tc.tile_pool(name="xload", bufs=7))
        work = ctx.enter_context(tc.tile_pool(name="work", bufs=3))
        ps_pj = ctx.enter_context(tc.tile_pool(name="ps_pj", bufs=2, space="PSUM"))
        ps_s = ctx.enter_context(tc.tile_pool(name="ps_s", bufs=2, space="PSUM"))
        ps_o = ctx.enter_context(tc.tile_pool(name="ps_o", bufs=2, space="PSUM"))

        wq_sb = const.tile([128, EC, 128], F16, tag="wq")
        wk_sb = const.tile([128, EC, 128], F16, tag="wk")
        wv_sb = const.tile([128, EC, 128], F16, tag="wv")
        wo_sb = const.tile([128, E], F16, tag="wo")
        bq_sb = const.tile([128, 1], F32, tag="bq")
        id_sb = const.tile([128, 128], F16, tag="id")
        # DMA priority order: first Q-proj needs only wq+bq; wk/wv before
        # the first kv quantum; wo/id not until the first consumer phase.
        nc.gpsimd.dma_start(wq_sb[:], wqt[:])
        nc.sync.dma_start(bq_sb[:], bqv[:])

        for rep in range(n_reps):
            qt_sb = persist.tile([128, Q_ROWS], F16, tag="qt", name=f"qt_{rep}")
            kt_sb = [persist.tile([128, SKV], F16, tag=f"kt{b}",
                                  name=f"kt{b}_{rep}") for b in range(B)]
            # v_sb[b][kv, kc, h*65 + d]; column h*65+64 is ones (denominator)
            v_sb = [persist.tile([128, KVC_B, 130], F16, tag=f"v{b}",
                                 name=f"v{b}_{rep}") for b in range(B)]
            at_sb = [persist.tile([128, SQ], F16, tag=f"at{b}",
                                  name=f"atz{b}_{rep}") for b in range(B)]

            for b in range(B):
                for h in range(2):
                    c = h * 65 + 64
                    nc.vector.memset(v_sb[b][:, :, c:c + 1], 1.0)

            def proj_q(j, u):
                xt = xload.tile([128, EC, 256], F16, tag="x",
                                name=f"xq{j}_{u}_{rep}")
                nc.gpsimd.dma_start(xt[:], x1t[j][:, :, u * 256:(u + 1) * 256])
                if not do_proj:
                    return
                q_ps = ps_pj.tile([128, 256], F32, tag="pj",
                                  name=f"qps{j}_{u}_{rep}")
                for ec in range(EC):
                    nc.tensor.matmul(q_ps[:], wq_sb[:, ec], xt[:, ec],
                                     start=(ec == 0), stop=(ec == EC - 1))
                c0 = j * 512 + u * 256
                nc.vector.tensor_scalar_add(qt_sb[:, c0:c0 + 256],
                                            q_ps[:], bq_sb[:])

            def proj_kv(b, jj, u):
                # one quantum: 256 kv tokens of batch b -> K.T rows + V chunks
                j = b * (SKV // 512) + jj
                xt = xload.tile([128, EC, 256], F16, tag="x",
                                name=f"xt{b}_{jj}_{u}_{rep}")
                nc.sync.dma_start(xt[:], x2t[j][:, :, u * 256:(u + 1) * 256])
                if not do_proj:
                    return
                k_ps = ps_pj.tile([128, 256], F32, tag="pj",
                                  name=f"kps{b}_{jj}_{u}_{rep}")
                for ec in range(EC):
                    nc.tensor.matmul(k_ps[:], wk_sb[:, ec], xt[:, ec],
                                     start=(ec == 0), stop=(ec == EC - 1))
                c0 = jj * 512 + u * 256
                nc.vector.tensor_copy(kt_sb[b][:, c0:c0 + 256], k_ps[:])
                # V in natural [kv, d] layout: lhsT = x2.T chunk (tokens move)
                for t in range(2):
                    kc = jj * 4 + u * 2 + t
                    v_ps = ps_pj.tile([128, 128], F32, tag="pj",
                                      name=f"vps{b}_{kc}_{rep}")
                    for ec in range(EC):
                        nc.tensor.matmul(
                            v_ps[:], xt[:, ec, t * 128:(t + 1) * 128],
                            wv_sb[:, ec],
                            start=(ec == 0), stop=(ec == EC - 1))
                    dst = v_sb[b][:, kc].rearrange("p (h x) -> p h x", h=2)
                    nc.vector.tensor_copy(
                        dst[:, :, 0:64],
                        v_ps[:].rearrange("p (h x) -> p h x", h=2))

            def scores_tile(b, g, h, kc, pt_full):
                gs0 = b * SQ + g * 512
                hp = h * 64
                s_ps = ps_s.tile([128, 1024], F32, tag="s",
                                 name=f"sps{b}_{g}_{kc}_{h}_{rep}")
                pt = work.tile([128, 1024], F16, tag="pt", bufs=34,
                               name=f"pt{b}_{g}_{kc}_{h}_{rep}")
                for u in range(2):
                    nc.tensor.matmul(
                        s_ps[:, u * 512:(u + 1) * 512],
                        kt_sb[b][hp:hp + 64,
                                 (kc + u) * 128:(kc + u + 1) * 128],
                        qt_sb[hp:hp + 64, gs0:gs0 + 512],
                        start=True, stop=True)
                nc.scalar.activation(pt[:], s_ps[:], Exp, scale=0.125)
                pt_full[(h, kc)] = pt

            def osweep_h(b, g, pt_full, anat, h, qc):
                # single-head accumulation group (final-phase pull-in)
                o2 = ps_o.tile([128, 65], F32, tag="o",
                               name=f"oh{b}_{g}_{h}_{qc}_{rep}")
                for kvc in range(KVC_B):
                    pt = pt_full[(h, kvc & ~1)]
                    c0 = (kvc & 1) * 512 + qc * 128
                    nc.tensor.matmul(
                        o2[:], pt[:, c0:c0 + 128],
                        v_sb[b][:, kvc, h * 65:h * 65 + 65],
                        start=(kvc == 0), stop=(kvc == KVC_B - 1))
                rec = work.tile([128, 1], F32, tag="rech", bufs=3,
                                name=f"rch{b}_{g}_{h}_{qc}_{rep}")
                nc.vector.reciprocal(rec[:], o2[:, 64:65])
                nc.vector.tensor_scalar_mul(
                    anat[qc][:, h * 64:(h + 1) * 64], o2[:, 0:64], rec[:])

            def trans_qc(b, g, anat, qc):
                atp = ps_pj.tile([128, 128], F16, tag="pj",
                                 name=f"atpf{b}_{g}_{qc}_{rep}")
                nc.tensor.transpose(atp[:], anat[qc][:], id_sb[:])
                nc.vector.tensor_copy(
                    at_sb[b][:, g * 512 + qc * 128:g * 512 + (qc + 1) * 128],
                    atp[:])

            def make_consumers(b, g, pt_full):
                # closures emitting the post-exp work of phase (b, g):
                # 4 merged-head o-sweeps (contiguous accum groups), 1
                # transpose step, 8 out-projection chunks. Executed during
                # phase (b,g)+1.
                anat = [work.tile([128, 128], F16, tag="anat", bufs=8,
                                  name=f"an{b}_{g}_{qc}_{rep}")
                        for qc in range(4)]

                def osweep(qc):
                    # both heads in ONE accumulation group / PSUM bank:
                    # h1's first matmul has start=False but its columns'
                    # has_written bits are clear (h0's start cleared the
                    # bank), so it overwrites, then accumulates.
                    o2 = ps_o.tile([128, 130], F32, tag="o",
                                   name=f"o{b}_{g}_{qc}_{rep}")
                    for h in range(2):
                        for kvc in range(KVC_B):
                            pt = pt_full[(h, kvc & ~1)]
                            c0 = (kvc & 1) * 512 + qc * 128
                            nc.tensor.matmul(
                                o2[:, h * 65:h * 65 + 65],
                                pt[:, c0:c0 + 128],
                                v_sb[b][:, kvc, h * 65:h * 65 + 65],
                                start=(h == 0 and kvc == 0),
                                stop=(h == 1 and kvc == KVC_B - 1))
                    rec = work.tile([128, 2], F32, tag="rec", bufs=3,
                                    name=f"rc{b}_{g}_{qc}_{rep}")
                    o2v = o2.rearrange("p (h x) -> p h x", x=65)
                    nc.vector.reciprocal(rec[:], o2v[:, :, 64])
                    for h in range(2):
                        nc.vector.tensor_scalar_mul(
                            anat[qc][:, h * 64:(h + 1) * 64],
                            o2[:, h * 65:h * 65 + 64], rec[:, h:h + 1])

                def at_step():
                    for qc in range(4):
                        atp = ps_pj.tile([128, 128], F16, tag="pj",
                                         name=f"atp{b}_{g}_{qc}_{rep}")
                        nc.tensor.transpose(atp[:], anat[qc][:], id_sb[:])
                        nc.vector.tensor_copy(
                            at_sb[b][:, g * 512 + qc * 128:
                                     g * 512 + (qc + 1) * 128],
                            atp[:])

                def oproj_o(o):
                    y_ps = ps_pj.tile([128, 512], F32, tag="pj",
                                      name=f"yps{b}_{g}_{o}_{rep}")
                    nc.tensor.matmul(y_ps[:], wo_sb[:, o * 128:(o + 1) * 128],
                                     at_sb[b][:, g * 512:(g + 1) * 512],
                                     start=True, stop=True)
                    y_sb = work.tile([128, 512], F16, tag="y", bufs=8,
                                     name=f"ysb{b}_{g}_{o}_{rep}")
                    # last phase: alternate copies onto Act so the final
                    # drain isn't serialized on DVE
                    if b == B - 1 and g == 1 and (o & 1):
                        nc.scalar.copy(y_sb[:], y_ps[:])
                    else:
                        nc.vector.tensor_copy(y_sb[:], y_ps[:])
                    nc.sync.dma_start(
                        yt_r[:, o, b * SQ + g * 512: b * SQ + (g + 1) * 512],
                        y_sb[:])

                cons = [(lambda qc=qc: osweep(qc)) for qc in range(4)]
                cons.append(at_step)
                if do_oproj:
                    cons += [(lambda o=o: oproj_o(o)) for o in range(EC)]
                return cons

            def attn_phase(b, g, consumers, fill, final=False):
                # interleave: previous phase's consumers + projection quanta
                # ride inside this phase's scores/exp stream
                if not do_attn:
                    for f in list(consumers) + list(fill):
                        f()
                    return []
                consumers = list(consumers)
                fill = list(fill)
                pt_full = {}
                anat_f = [work.tile([128, 128], F16, tag="anat", bufs=8,
                                    name=f"anf{b}_{g}_{qc}_{rep}")
                          for qc in range(4)] if final else None
                pulled = 0
                ci = fi = 0
                for ip in range(16):
                    h, kp = divmod(ip, 8)
                    while ci < len(consumers) and \
                            ci * 16 < (ip + 1) * len(consumers):
                        consumers[ci]()
                        ci += 1
                    while fi < len(fill) and fi * 12 < (ip + 1) * len(fill):
                        fill[fi]()
                        fi += 1
                    # final phase: h0 sweeps ride inside the h1 score tiles
                    if final and ip >= 8 and (ip - 8) % 2 == 0 and pulled < 4:
                        osweep_h(b, g, pt_full, anat_f, 0, pulled)
                        pulled += 1
                    scores_tile(b, g, h, kp * 2, pt_full)
                for c in consumers[ci:]:
                    c()
                for f in fill[fi:]:
                    f()
                if not final:
                    return make_consumers(b, g, pt_full)
                # final drain: h1 sweeps chained with per-qc transposes,
                # then out-projection with copies split across DVE/Act
                while pulled < 4:
                    osweep_h(b, g, pt_full, anat_f, 0, pulled)
                    pulled += 1
                osweep_h(b, g, pt_full, anat_f, 1, 0)
                osweep_h(b, g, pt_full, anat_f, 1, 1)
                trans_qc(b, g, anat_f, 0)
                osweep_h(b, g, pt_full, anat_f, 1, 2)
                trans_qc(b, g, anat_f, 1)
                osweep_h(b, g, pt_full, anat_f, 1, 3)
                trans_qc(b, g, anat_f, 2)
                trans_qc(b, g, anat_f, 3)
                if do_oproj:
                    # batch the last outputs into two 4-chunk DMAs so the
                    # drain isn't paced by per-DMA issue overhead
                    for half in range(2):
                        yw = work.tile([128, 4, 512], F16, tag="ywide",
                                       bufs=2, name=f"ywf{half}_{rep}")
                        for oo in range(4):
                            o = half * 4 + oo
                            pool, tg = (ps_pj, "pj") if o % 2 == 0 else (ps_s, "s")
                            y_ps = pool.tile([128, 512], F32, tag=tg,
                                             name=f"ypsf{o}_{rep}")
                            nc.tensor.matmul(
                                y_ps[:], wo_sb[:, o * 128:(o + 1) * 128],
                                at_sb[b][:, g * 512:(g + 1) * 512],
                                start=True, stop=True)
                            if o & 1:
                                nc.scalar.copy(yw[:, oo], y_ps[:])
                            else:
                                nc.vector.tensor_copy(yw[:, oo], y_ps[:])
                        nc.sync.dma_start(
                            yt_r[:, half * 4:(half + 1) * 4,
                                 b * SQ + g * 512: b * SQ + (g + 1) * 512],
                            yw[:])
                return []

            # lead-in: projections for batch 0 (+ first two q chunks).
            # Q quanta lead (x1 arrives on the parallel gpsimd channel);
            # the x2/kv chain is DMA-paced, so its first load goes early
            # in the sync queue (right after wq+wk).
            nc.sync.dma_start(wk_sb[:], wkt[:])
            lead = [(lambda u=u: proj_q(0, u)) for u in range(2)]
            lead += [(lambda u=u: proj_q(1, u)) for u in range(2)]
            kvq = [(lambda jj=jj, u=u: proj_kv(0, jj, u))
                   for jj in range(2) for u in range(2)]
            order = [lead[0], lead[1], kvq[0], kvq[1], lead[2], kvq[2],
                     lead[3], kvq[3]]
            for i, qm in enumerate(order):
                if i == 2:
                    nc.sync.dma_start(wv_sb[:], wvt[:])
                qm()
                if i == 4:
                    nc.sync.dma_start(wo_sb[:], wot[:])
                    nc.sync.dma_start(id_sb[:], idv[:])

            cons = []
            for b in range(B):
                for g in range(2):
                    # every phase carries ~8.5us of projection quanta:
                    # (b,0) runs batch b's OWN back-half KV projection
                    # (score tiles consume kt chunks progressively in kc
                    # order, so same-phase production is safe) + next q
                    # chunk; (b,1) runs batch b+1's front-half KV.
                    fl = []
                    if g == 0:
                        fl += [(lambda jj=jj, u=u: proj_kv(b, jj, u))
                               for jj in range(2, 4) for u in range(2)]
                        if b + 1 < B:
                            fl += [(lambda j=2 * b + 2, u=u: proj_q(j, u))
                                   for u in range(2)]
                        else:
                            fl += [(lambda u=u: proj_q(2 * B - 1, u))
                                   for u in range(2)]
                    elif b + 1 < B:
                        fl += [(lambda jj=jj, u=u: proj_kv(b + 1, jj, u))
                               for jj in range(2) for u in range(2)]
                        fl += [(lambda j=2 * b + 3, u=u: proj_q(j, u))
                               for u in range(2)]
                    cons = attn_phase(b, g, cons, fl,
                                      final=(b == B - 1 and g == 1))
            for c in cons:
                c()

    nc.compile()
    return nc


def _get_nc(phases=("proj", "attn", "oproj"), n_reps=1):
    key = (tuple(phases), n_reps)
    if key not in _CACHE:
        _CACHE[key] = _build(phases, n_reps)
    return _CACHE[key]


def _tile_x(xt2d, nchunks):
    # [E, R] -> [R/512, 128, EC, 512]: x[j, p, ec, q] = xt2d[ec*128+p, j*512+q]
    return np.ascontiguousarray(
        xt2d.reshape(EC, 128, nchunks, 512).transpose(2, 1, 0, 3))


def _tile_w(wt_slice):
    # [E, 128] -> [128, EC, 128]
    return np.ascontiguousarray(
        wt_slice.reshape(EC, 128, 128).transpose(1, 0, 2))


def make_in_maps(x1, x2, Wq, bq, Wk, bk, Wv, bv, Wo, bo=None):
    f16 = np.float16
    x1 = np.asarray(x1, dtype=np.float32)
    x2 = np.asarray(x2, dtype=np.float32)
    x1t = _tile_x(np.ascontiguousarray(x1.reshape(Q_ROWS, E).T), QC).astype(f16)
    x2t = _tile_x(np.ascontiguousarray(x2.reshape(KV_ROWS, E).T),
                  KV_ROWS // 512).astype(f16)
    WqT = np.asarray(Wq, dtype=np.float32).T
    WkT = np.asarray(Wk, dtype=np.float32).T
    WvT = np.asarray(Wv, dtype=np.float32).T
    WoT = np.ascontiguousarray(np.asarray(Wo, dtype=np.float32).T)
    ident = np.eye(128, dtype=f16)
    in_maps = []
    for c in range(N_CORES):
        s = slice(128 * c, 128 * (c + 1))
        in_maps.append({
            "x1t": x1t, "x2t": x2t,
            "wqt": _tile_w(WqT[:, s]).astype(f16),
            "wkt": _tile_w(WkT[:, s]).astype(f16),
            "wvt": _tile_w(WvT[:, s]).astype(f16),
            "wot": np.ascontiguousarray(WoT[s, :]).astype(f16),
            "bq": np.ascontiguousarray(
                np.asarray(bq, np.float32)[s]).reshape(128, 1),
            "ident": ident,
        })
    return in_maps


def kernel(x1, x2, Wq, bq, Wk, bk, Wv, bv, Wo, bo):
    nc = _get_nc()
    in_maps = make_in_maps(x1, x2, Wq, bq, Wk, bk, Wv, bv, Wo)
    res = run_bass_kernel_spmd(nc, in_maps, list(range(N_CORES)))
    ytf = res.results[0]["yt"].astype(np.float32)
    for c in range(1, N_CORES):
        ytf += res.results[c]["yt"].astype(np.float32)
    # bv folds into the output bias: softmax rows sum to 1
    bo_eff = (np.asarray(bo, np.float64)
              + np.asarray(Wo, np.float64) @ np.asarray(bv, np.float64))
    y = ytf.T.astype(np.float32) + bo_eff.astype(np.float32)[None, :]
    return y.reshape(B, SQ, E)
